# revision 4
# baseline (speedup 1.0000x reference)
"""TRN2 Bass kernel for nn_CrossModalAttention_75316546503126.

Mathematical collapse exploited here (verified against the jax reference):
K/V rows of the attention are identical across the sequence axis because the
acoustic features are broadcast before the K/V projections.  Hence every
attention row sees a constant score vector, softmax is exactly uniform
(S = 2048 is a power of two, so 1/S is exact in fp32), and

    out[b, s, :] = text[b, s, :] @ Wt + bias_b
    bias_b       = bias0 + ac_b @ Wav
    Wav          = Wa @ Wv,  bias0 = bt + bv + ba @ Wv   (weight-only fusions
                                                          done on the host)

i.e. one [S, D] x [D, D] matmul per batch plus a per-batch bias row.  All
data-dependent compute (text @ Wt, ac @ Wav) runs on device.

Sharding: data-parallel over batch B=8 across the 8 NeuronCores.

v4 implementation (Wt-stationary, transposed output, phase-split):
  - Computes out^T = Wt^T @ X^T: each 128x128 Wt block is the PE stationary
    operand and streams 512-column X^T chunks; 144 N=512 matmuls run at the
    measured 215 ns cadence (full 2.4 GHz, LDWEIGHTS fully pipelined).
  - Phase split keeps the PE comfortably BEHIND the DMA stream (pacing them
    1:1 lets micro-stalls reset the HAM clock ramp to 1.2 GHz):
      P1: d-blocks 0..3 x s-chunks 0,1  -- first-half slabs + d0..3 weight
          columns only (~0.13 MB per k-slice), 8 x [128,512] PSUM banks.
      P2: d-blocks 0..3 x s-chunks 2,3  -- second-half slabs, all resident.
      P3: d4 (k-outer), then d5 emitted k-INNER per chunk so each chunk's
          accumulation group closes ~1.3us apart and the four evict+store
          steps pipeline against the last matmuls (short tail drain).
  - The per-batch bias is pure DVE work: 6 scalar_tensor_tensor ops with
    accum_out reduce Wav^T * ac (K=16) per d-block, plus the host-folded
    bias0^T; eviction adds it via per-partition tensor_scalar_add (bias
    varies along d = partitions in the transposed layout) and downcasts to
    bf16.  The host transposes the output back.
  - WARMUP_MM dummy matmuls (one long PSUM accumulation group) bridge the
    ~8.7us DGE-dead window so the PE clock is ramped when real work starts.
"""
import sys

if "/opt/trn_rl_repo" not in sys.path:
    sys.path.insert(0, "/opt/trn_rl_repo")

from contextlib import ExitStack

import numpy as np
import ml_dtypes

import concourse.bacc as bacc
import concourse.bass as bass
import concourse.mybir as mybir
import concourse.tile as tile
from concourse.bass_utils import run_bass_kernel_spmd

F32 = mybir.dt.float32
BF16 = mybir.dt.bfloat16
MULT = mybir.AluOpType.mult

B, S, D = 8, 2048, 768
KB = D // 128           # 6 contraction blocks
DB = D // 128           # 6 output d-blocks
CS = 512                # psum-bank chunk of the s axis
HS = S // 2             # half-slab columns
N_CORES = 8

WARMUP_MM = 6           # dummy [128,512] matmuls that ramp the PE clock

MODE = "bf16"


def build_program(mode=MODE):
    nc = bacc.Bacc()

    # xt[p, k*S + s] = X[s, k*128+p]   (k-major X^T slabs)
    xt = nc.declare_dram_parameter("xt", [128, KB * S], BF16, isOutput=False)
    # wt region A (cols [0,3072)):  [k*512 + d]        = Wt[k*128+p, d], d<512
    # wt region B (cols [3072,4608)): [3072+k*256+d']  = Wt[k*128+p, 512+d']
    wt = nc.declare_dram_parameter("wt", [128, KB * D], BF16, isOutput=False)
    # wavT2[p, db*16+q] = Wav[q, db*128+p];  acB[p, q] = ac[q]
    wavT2 = nc.declare_dram_parameter("wavT2", [128, DB * 16], F32,
                                      isOutput=False)
    acB = nc.declare_dram_parameter("acB", [128, 16], F32, isOutput=False)
    b0t = nc.declare_dram_parameter("b0t", [128, DB], F32, isOutput=False)
    outT = nc.declare_dram_parameter("outT", [D, S], BF16, isOutput=True)

    with tile.TileContext(nc) as tc, ExitStack() as ctx:
        const = ctx.enter_context(tc.tile_pool(name="const", bufs=1))
        wpool = ctx.enter_context(tc.tile_pool(name="wpool", bufs=1))
        xpool = ctx.enter_context(tc.tile_pool(name="xpool", bufs=1))
        opool = ctx.enter_context(tc.tile_pool(name="opool", bufs=3))
        # PSUM: 8 x [128, 512] f32 -- one bank per tile
        pso = ctx.enter_context(tc.tile_pool(name="pso", bufs=8, space="PSUM"))

        # ---------------- PE warm-up fodder (no DMA dependencies) --------
        warm_w = const.tile([128, 128], BF16)
        nc.gpsimd.memset(warm_w[:], 1.0)
        warm_x = const.tile([128, 512], BF16)
        nc.gpsimd.memset(warm_x[:], 1.0)

        # ---------------- DMA schedule ----------------
        # scalar queue: tiny bias tensors first, output stores later
        acB_sb = const.tile([128, 16], F32)
        nc.scalar.dma_start(acB_sb[:], acB[:])
        wavT2_sb = const.tile([128, DB * 16], F32)
        nc.scalar.dma_start(wavT2_sb[:], wavT2[:])
        b0t_sb = const.tile([128, DB], F32)
        nc.scalar.dma_start(b0t_sb[:], b0t[:])

        # sync queue: (wt d0..3 k-slice, X^T k-slab first half) pairs, then
        # the second halves, then the d4/d5 weight columns (needed by P3)
        wtka = []
        xh = [[None, None] for _ in range(KB)]
        for k in range(KB):
            t = wpool.tile([128, 512], BF16, tag=f"wt{k}", name=f"wt{k}")
            nc.sync.dma_start(t[:], wt[:, k * 512:(k + 1) * 512])
            wtka.append(t)
            xtile = xpool.tile([128, HS], BF16, tag=f"x{k}0", name=f"x{k}0")
            nc.sync.dma_start(xtile[:], xt[:, k * S:k * S + HS])
            xh[k][0] = xtile
        for k in range(KB):
            xtile = xpool.tile([128, HS], BF16, tag=f"x{k}1", name=f"x{k}1")
            nc.sync.dma_start(xtile[:], xt[:, k * S + HS:(k + 1) * S])
            xh[k][1] = xtile
        wtb = wpool.tile([128, KB * 256], BF16, tag="wtb", name="wtb")
        nc.sync.dma_start(wtb[:], wt[:, 3072:4608])

        def wt_block(k, db):
            if db < 4:
                return wtka[k][:, db * 128:(db + 1) * 128]
            return wtb[:, k * 256 + (db - 4) * 128:k * 256 + (db - 3) * 128]

        def x_chunk(k, c):
            return xh[k][c // 2][:, (c % 2) * CS:(c % 2) * CS + CS]

        # ---------------- PE warm-up (DMA-dead window) -------------------
        warm_ps = pso.tile([128, CS], F32, tag="po", name="warm_ps")
        for i in range(WARMUP_MM):
            nc.tensor.matmul(warm_ps[:], warm_w[:], warm_x[:],
                             start=(i == 0), stop=(i == WARMUP_MM - 1),
                             skip_group_check=True)

        # ---------------- bias on DVE only: biasT = b0t + Wav^T ac^T -----
        junk = const.tile([128, 16], F32)
        braw = const.tile([128, DB], F32)
        for db in range(DB):
            nc.vector.scalar_tensor_tensor(
                junk[:], wavT2_sb[:, db * 16:(db + 1) * 16], 1.0, acB_sb[:],
                MULT, MULT, accum_out=braw[:, db:db + 1])
        biasT = const.tile([128, DB], F32)
        nc.vector.tensor_add(biasT[:], braw[:], b0t_sb[:])

        # ---------------- main emitters ----------------
        def evict_pair(db, ps_lo, ps_hi, half):
            # half: 0 -> s columns [0,1024), 1 -> [1024,2048)
            ot = opool.tile([128, 2 * CS], BF16, tag="o")
            nc.vector.tensor_scalar_add(ot[:, 0:CS], ps_lo[:],
                                        biasT[:, db:db + 1])
            nc.vector.tensor_scalar_add(ot[:, CS:2 * CS], ps_hi[:],
                                        biasT[:, db:db + 1])
            nc.scalar.dma_start(
                outT[db * 128:(db + 1) * 128, half * HS:(half + 1) * HS],
                ot[:])

        # P1: d0..d3 on s-chunks 0,1 -- slab-paced, interleaved per k
        psA = [[pso.tile([128, CS], F32, tag="po", name=f"p1d{d}c{c}")
                for c in range(2)] for d in range(4)]
        for k in range(KB):
            for d in range(4):
                w = wt_block(k, d)
                for c in range(2):
                    nc.tensor.matmul(psA[d][c][:], w, x_chunk(k, c),
                                     start=(k == 0), stop=(k == KB - 1))
        for d in range(4):
            evict_pair(d, psA[d][0], psA[d][1], 0)

        # P2: d0..d3 on s-chunks 2,3 -- solo per d-block (staggered evicts)
        for d in range(4):
            ps = [pso.tile([128, CS], F32, tag="po", name=f"p2d{d}c{c}")
                  for c in range(2)]
            for k in range(KB):
                w = wt_block(k, d)
                for c in range(2):
                    nc.tensor.matmul(ps[c][:], w, x_chunk(k, 2 + c),
                                     start=(k == 0), stop=(k == KB - 1))
            evict_pair(d, ps[0], ps[1], 1)

        # P3: d4 (k-outer), then d5 k-INNER per chunk -> staggered group
        # closes; evict+store per chunk pipelines against the last matmuls
        ps4 = [pso.tile([128, CS], F32, tag="po", name=f"p3d4c{c}")
               for c in range(4)]
        for k in range(KB):
            w = wt_block(k, 4)
            for c in range(4):
                nc.tensor.matmul(ps4[c][:], w, x_chunk(k, c),
                                 start=(k == 0), stop=(k == KB - 1))
        evict_pair(4, ps4[0], ps4[1], 0)
        evict_pair(4, ps4[2], ps4[3], 1)

        store_eng = [nc.sync, nc.scalar, nc.sync, nc.scalar]
        for c in range(4):
            ps = pso.tile([128, CS], F32, tag="po", name=f"p3d5c{c}")
            for k in range(KB):
                nc.tensor.matmul(ps[:], wt_block(k, 5), x_chunk(k, c),
                                 start=(k == 0), stop=(k == KB - 1))
            ot = opool.tile([128, CS], BF16, tag="o5", bufs=2)
            nc.vector.tensor_scalar_add(ot[:], ps[:], biasT[:, 5:6])
            store_eng[c].dma_start(
                outT[5 * 128:6 * 128, c * CS:(c + 1) * CS], ot[:])

    nc.compile()
    return nc


_PROGRAM_CACHE = {}


def _get_program(mode=None):
    if mode is None:
        mode = MODE
    if mode not in _PROGRAM_CACHE:
        _PROGRAM_CACHE[mode] = build_program(mode)
    return _PROGRAM_CACHE[mode]


def make_in_maps(text_features, acoustic_features, Wt, bt, Wa, ba, Wv, bv):
    """Host-side sharding + layout prep: per-batch X^T k-slabs, k-major Wt
    slices (d0..3 | d4..5 split), and the host-fused bias terms
    (Wav = Wa @ Wv, bias0 = bt + bv + ba @ Wv)."""
    bf16 = ml_dtypes.bfloat16
    text_features = np.asarray(text_features, dtype=np.float32)
    # xt[b, p, k*S + s] = X[b, s, k*128+p]
    xt_all = (text_features
              .reshape(B, S, KB, 128)
              .transpose(0, 3, 2, 1)
              .astype(bf16)
              .reshape(B, 128, KB * S))

    Wt3 = np.asarray(Wt, dtype=np.float32).reshape(KB, 128, D)
    wtA = Wt3[:, :, 0:512].transpose(1, 0, 2).reshape(128, KB * 512)
    wtB = Wt3[:, :, 512:768].transpose(1, 0, 2).reshape(128, KB * 256)
    wt_packed = np.concatenate([wtA, wtB], axis=1).astype(bf16)

    Wa = np.asarray(Wa, dtype=np.float32)
    Wv = np.asarray(Wv, dtype=np.float32)
    wav = Wa @ Wv                                   # [16, D]
    wavT2 = np.ascontiguousarray(
        wav.reshape(16, DB, 128).transpose(2, 1, 0).reshape(128, DB * 16))
    bias0 = (np.asarray(bt, dtype=np.float32)
             + np.asarray(bv, dtype=np.float32)
             + np.asarray(ba, dtype=np.float32) @ Wv)
    b0t = np.ascontiguousarray(bias0.reshape(DB, 128).T)

    shared = {
        "wt": np.ascontiguousarray(wt_packed),
        "wavT2": wavT2,
        "b0t": b0t,
    }
    acoustic_features = np.asarray(acoustic_features, dtype=np.float32)
    in_maps = []
    for b in range(N_CORES):
        m = dict(shared)
        m["xt"] = np.ascontiguousarray(xt_all[b])
        m["acB"] = np.ascontiguousarray(
            np.broadcast_to(acoustic_features[b], (128, 16)))
        in_maps.append(m)
    return in_maps


def kernel(text_features, acoustic_features, Wt, bt, Wa, ba, Wq, bq, Wk, bk,
           Wv, bv, **_unused):
    nc = _get_program()
    in_maps = make_in_maps(text_features, acoustic_features, Wt, bt, Wa, ba,
                           Wv, bv)
    res = run_bass_kernel_spmd(nc, in_maps, list(range(N_CORES))).results
    out = np.empty((B, S, D), dtype=np.float32)
    for b in range(N_CORES):
        out[b] = np.asarray(res[b]["outT"], dtype=np.float32).T
    return out
